# revision 16
# baseline (speedup 1.0000x reference)
"""Trainium2 Bass kernel for DiffAttention (nn_DiffAttention_49847390437777).

Contract: kernel(**full_inputs) -> full output [2, 2048, 8, 256] fp32.

Sharding (8 cores): core c handles batch b = c//4 and global query-head pairs
{2*(c%4), 2*(c%4)+1} (i.e. heads 4*(c%4)..4*(c%4)+3).  Diff-attention couples
only adjacent head pairs, which stay co-located.  lambda is computed on host
and baked into the program as an immediate; subln_weight is applied on host
after the gather (it multiplies AFTER the RMS norm, so this is exact).

Device algorithm per core (4 heads = 2 pairs, seq 2048, head_dim 128):
  - scores transposed: S^T[k, q] = kT_blk.T @ qT_blk; causal blocks only.
  - softmax without max-subtraction; rowsum folded into the PV matmul via a
    ones-column appended to V (psum col 256).
  - the first 512 q rows run in float32r (cancellation there amplifies bf16
    noise through the RMS norm).  All f32r matmuls keep output free size
    >= 256 so they run at full 1 cycle/row PE speed (the diagonal kb3 block
    is widened from 128 to 256 cols; the extra cols are zeroed in P after).
  - epilogue per 128-row block:  negE = (O2*mu) - O1 with mu = lam*s1/s2
    (one fused scalar_tensor_tensor from PSUM), ssq = sum(negE^2) (fused
    square+row-reduce), then rms scale via ACT Ln/Exp:
       out = negE * (-r1 * exp(-0.5*ln(ssq*r1^2/256 + eps) + ln(1-li)))
  - program order interleaves QK score matmuls of step i+1 with PV matmuls
    of step i so the PE never stalls on the exp drain; the pipeline runs
    across both head pairs (no drain at the pair boundary).  Step order is
    qb = 1,2,3,0 per pair so the fp32 input DMA is off the critical path and
    the final tail (PV of qb0) is the smallest.
  - output is stored bf16 (post-RMS values; rounding is ~0.4% of unit scale)
    and converted to fp32 on host.
"""

import math
import os

import numpy as np
import ml_dtypes

HEAD_DIM = 128
N_HEADS = 16
LAYER_IDX = 12
LAMBDA_INIT = 0.8 - 0.6 * math.exp(-0.3 * (LAYER_IDX - 1))
EPS = 1e-5
SCALE = 1.0 / math.sqrt(HEAD_DIM)
S_FOLD = 1.0 - LAMBDA_INIT

B = 2
S = 2048
NB = S // 128   # 16 key blocks of 128
QB = S // 512   # 4 query superblocks of 512
N_CORES = 8

bf16 = ml_dtypes.bfloat16

_CACHE = {}
last_results = None  # BassKernelResults of the most recent run (for test.py)


def build_nc(lam_full, hiprec=True):
    """Build + compile the per-core Bass program (same program on all cores)."""
    import concourse.bass as bass
    import concourse.mybir as mybir
    import concourse.bacc as bacc
    import concourse.tile as tile
    from concourse.masks import make_upper_triangular
    from contextlib import ExitStack

    f32 = mybir.dt.float32
    f32r = mybir.dt.float32r
    b16 = mybir.dt.bfloat16
    AF = mybir.ActivationFunctionType
    ALU = mybir.AluOpType
    LAM = float(lam_full)

    nc = bacc.Bacc("TRN2", target_bir_lowering=False, debug=False)

    hp32 = nc.dram_tensor("hp32", [2, 128, 2, 1024], f32r, kind="ExternalInput")
    qkb = nc.dram_tensor("qkb", [2, 2, 128, 4096], b16, kind="ExternalInput")
    vxb = nc.dram_tensor("vxb", [2, 128, NB, 257], b16, kind="ExternalInput")
    vx32 = nc.dram_tensor("vx32", [2, 128, 4, 260], f32r, kind="ExternalInput")
    o = nc.dram_tensor("o", [2, 128, NB, 256], b16, kind="ExternalOutput")

    with tile.TileContext(nc) as tc:
        with ExitStack() as ctx:
            ec = ctx.enter_context
            const = ec(tc.tile_pool(name="const", bufs=1))
            qkpool = ec(tc.tile_pool(name="qkpool", bufs=2))
            hppool = ec(tc.tile_pool(name="hppool", bufs=2))
            vpool = ec(tc.tile_pool(name="vpool", bufs=2))
            v32pool = ec(tc.tile_pool(name="v32pool", bufs=2))
            ppool = ec(tc.tile_pool(name="ppool", bufs=3))
            p3pool = ec(tc.tile_pool(name="p3pool", bufs=1))
            negepool = ec(tc.tile_pool(name="negepool", bufs=4))
            stat = ec(tc.tile_pool(name="stat", bufs=4))
            sqpool = ec(tc.tile_pool(name="sqpool", bufs=2))
            otpool = ec(tc.tile_pool(name="otpool", bufs=2))
            spsum = ec(tc.tile_pool(name="spsum", bufs=2, space="PSUM"))
            opsum = ec(tc.tile_pool(name="opsum", bufs=2, space="PSUM"))

            tri16 = const.tile([128, 128], b16)
            make_upper_triangular(nc, tri16[:], val=1.0, diag=True)
            tri32 = const.tile([128, 128], f32)
            make_upper_triangular(nc, tri32[:], val=1.0, diag=True)
            z16 = const.tile([128, 128], b16)
            make_upper_triangular(nc, z16[:], val=0.0, diag=False)
            z32 = const.tile([128, 128], f32)
            make_upper_triangular(nc, z32[:], val=0.0, diag=False)
            eps_t = const.tile([128, 1], f32)
            nc.gpsimd.memset(eps_t[:], EPS)
            lsf_t = const.tile([128, 1], f32)
            nc.gpsimd.memset(lsf_t[:], math.log(S_FOLD))

            # ---- input DMAs, ordered by need ----
            # One tile per DMA chunk: tile-granular dependency tracking means
            # a consumer waits every DMA into its tile, so chunks get own
            # tiles, sized so the first QK only waits ~0.5 MB.
            k_a, k_b, k_c, q_b, q_c = {}, {}, {}, {}, {}
            hp_t, vx_b, vx_3 = {}, {}, {}
            for pair in range(2):
                for par in range(2):   # k blocks 0-3 + its q superblock first
                    k_a[pair, par] = qkpool.tile(
                        [128, 512], b16, tag=f"k_a{par}", name="k_a")
                    nc.sync.dma_start(k_a[pair, par][:], qkb[pair, par, :, 2048:2560])
                    q_b[pair, par] = qkpool.tile(
                        [128, 512], b16, tag=f"q_b{par}", name="q_b")
                    nc.sync.dma_start(q_b[pair, par][:], qkb[pair, par, :, 512:1024])
                for par in range(2):   # fp32 q/k for the qb0 step (2nd step)
                    hp_t[pair, par] = hppool.tile(
                        [128, 1024], f32r, tag=f"hp{par}", name="hp_t")
                    nc.sync.dma_start(hp_t[pair, par][:], hp32[pair, :, par, :])
                vx_b[pair] = vpool.tile([128, NB, 257], b16, tag="vx", name="vx_b")
                nc.sync.dma_start(vx_b[pair][:], vxb[pair])
                for par in range(2):   # k blocks 4-7
                    k_b[pair, par] = qkpool.tile(
                        [128, 512], b16, tag=f"k_b{par}", name="k_b")
                    nc.sync.dma_start(k_b[pair, par][:], qkb[pair, par, :, 2560:3072])
                for par in range(2):   # q cols 1024:2048
                    q_c[pair, par] = qkpool.tile(
                        [128, 1024], b16, tag=f"q_c{par}", name="q_c")
                    nc.sync.dma_start(q_c[pair, par][:], qkb[pair, par, :, 1024:2048])
                for par in range(2):   # k blocks 8-15
                    k_c[pair, par] = qkpool.tile(
                        [128, 1024], b16, tag=f"k_c{par}", name="k_c")
                    nc.sync.dma_start(k_c[pair, par][:], qkb[pair, par, :, 3072:4096])
                vx_3[pair] = v32pool.tile([128, 4, 260], f32r, tag="vx32", name="vx_3")
                nc.sync.dma_start(vx_3[pair][:], vx32[pair])

            def k_src(pair, par, kb):
                if kb < 4:
                    return k_a[pair, par][:], kb * 128
                if kb < 8:
                    return k_b[pair, par][:], (kb - 4) * 128
                return k_c[pair, par][:], (kb - 8) * 128

            def q_src(pair, par, qcol):  # global q column -> (tile, col)
                if qcol < 1024:
                    return q_b[pair, par][:], qcol - 512
                return q_c[pair, par][:], qcol - 1024

            # ---- emission units ----
            def scores_units(pair, qb):
                """List of closures; each emits one (par, 2-kb group):
                QK matmuls -> exp -> tri mask / junk memset."""
                hi = (qb == 0) and hiprec
                q0 = qb * 512
                if hi:
                    pt = {par: p3pool.tile([128, 4, 512], f32r, tag=f"p3_{par}",
                                           name="p3") for par in range(2)}
                else:
                    pt = {par: ppool.tile([128, NB, 512], b16, tag=f"p1_{par}",
                                          name="p1") for par in range(2)}
                ngroups = 2 * qb + 2
                units = []

                def mk(par, g):
                    def emit():
                        dst = pt[par]
                        sp = spsum.tile([128, 2, 512], f32, tag="sp", name="sp")
                        kbs = (2 * g, 2 * g + 1)
                        qoffs = []
                        for t, kb in enumerate(kbs):
                            qoff = 0 if kb < 4 * qb else (kb - 4 * qb) * 128
                            if hi and kb == 3:
                                qoff = 256  # widen to keep f32r at 1 cyc/row
                            qoffs.append(qoff)
                            if hi:
                                kt = hp_t[pair, par][:, 512:1024]
                                ktcol = kb * 128
                                qt = hp_t[pair, par][:, 0:512]
                                qcol = q0 + qoff
                            else:
                                kt, ktcol = k_src(pair, par, kb)
                                qt, qcol = q_src(pair, par, q0 + qoff)
                            nc.tensor.matmul(
                                sp[:, t, qoff:512],
                                kt[:, ktcol:ktcol + 128],
                                qt[:, qcol:qcol + 512 - qoff],
                                start=True, stop=True,
                            )
                        # one exp instr over the group rectangle
                        lo = min(qoffs)
                        nc.scalar.activation(
                            dst[:, kbs[0]:kbs[0] + 2, lo:512],
                            sp[:, 0:2, lo:512],
                            AF.Exp, scale=SCALE,
                        )
                        # junk cols of the rectangle for the 2nd block
                        zt = z32 if hi else z16
                        if qoffs[1] > lo:
                            nc.vector.tensor_copy(
                                dst[:, kbs[1], lo:qoffs[1]],
                                zt[:, 0:qoffs[1] - lo])
                        if hi and kbs[1] == 3:
                            # widened matmul: [256:384) is above-diagonal
                            nc.vector.tensor_copy(
                                dst[:, 3, 256:384], zt[:, 0:128])
                        # triangular mask on diagonal blocks
                        for kb in kbs:
                            if kb >= 4 * qb:
                                dq = q0 + 0  # diag of kb at q col (kb*128)
                                col = (kb - 4 * qb) * 128
                                tri = tri32 if hi else tri16
                                nc.vector.tensor_mul(
                                    dst[:, kb, col:col + 128],
                                    dst[:, kb, col:col + 128], tri[:])
                    return emit

                for g in range(ngroups):
                    for par in range(2):
                        kbs = (2 * g, 2 * g + 1)
                        qoffs = [0 if kb < 4 * qb else
                                 (256 if hi and kb == 3 else (kb - 4 * qb) * 128)
                                 for kb in kbs]
                        pe = sum(512 - qo for qo in qoffs) / 2.4
                        act = 2 * (512 - min(qoffs)) * 0.833 + 265
                        units.append((mk(par, g), pe, act))
                return units, pt, hi

            def pv_units(pair, qb, pt, hi, tail=False):
                """5 closures: 4 j-runs (PV matmuls + DVE epilogue) + finalize.
                tail=True: runs after the last scores step - op tiles may also
                use the idle scores PSUM, and each j finalizes + DMAs itself
                so output overlaps the remaining PV matmuls."""
                rcq = stat.tile([128, 4, 2], f32, tag="rcq", name="rcq")
                muq = stat.tile([128, 4], f32, tag="muq", name="muq")
                ssq = stat.tile([128, 4], f32, tag="ssq", name="ssq")
                negE = negepool.tile([128, 4, 256], b16, tag="negE", name="negE")
                vx = vx_3[pair] if hi else vx_b[pair]
                ncols = 258 if hi else 257  # f32r matmul needs even free size
                units = []

                def mkj(j):
                    def emit():
                        jabs = 4 * qb + j
                        pool = spsum if (tail and j % 2) else opsum
                        opt = pool.tile([128, 2, 512], f32,
                                        tag="sp" if (tail and j % 2) else "op",
                                        name="opt")
                        for par in range(2):
                            for kb in range(jabs + 1):
                                nc.tensor.matmul(
                                    opt[:, par, 0:ncols],
                                    pt[par][:, kb, j * 128:(j + 1) * 128],
                                    vx[:, kb, 0:ncols],
                                    start=(kb == 0), stop=(kb == jabs),
                                )
                        # epilogue for this q block
                        nc.vector.reciprocal(rcq[:, j, :], opt[:, :, 256])
                        nc.vector.scalar_tensor_tensor(
                            muq[:, j:j + 1], opt[:, 0, 256:257], LAM,
                            rcq[:, j, 1:2], ALU.mult, ALU.mult)
                        # DVE may read only one non-scalar PSUM input per op
                        t2 = sqpool.tile([128, 256], f32, tag="t2", name="t2")
                        nc.vector.tensor_scalar_mul(
                            t2[:], opt[:, 1, 0:256], muq[:, j:j + 1])
                        nc.vector.tensor_sub(
                            negE[:, j, :], t2[:], opt[:, 0, 0:256])
                        sq = sqpool.tile([128, 256], b16, tag="sq", name="sq")
                        nc.vector.scalar_tensor_tensor(
                            sq[:], negE[:, j, :], 1.0, negE[:, j, :],
                            ALU.mult, ALU.mult, accum_out=ssq[:, j:j + 1])
                        if tail:
                            r1s = stat.tile([128, 1], f32, tag="r1sj", name="r1s")
                            nc.vector.tensor_mul(
                                r1s[:], rcq[:, j, 0:1], rcq[:, j, 0:1])
                            nc.vector.tensor_mul(r1s[:], r1s[:], ssq[:, j:j + 1])
                            lnj = stat.tile([128, 1], f32, tag="lnj", name="lnj")
                            nc.scalar.activation(lnj[:], r1s[:], AF.Ln,
                                                 scale=1.0 / 256.0, bias=eps_t[:])
                            nc.scalar.activation(lnj[:], lnj[:], AF.Exp,
                                                 scale=-0.5, bias=lsf_t[:])
                            csj = stat.tile([128, 1], f32, tag="csj", name="csj")
                            nc.vector.scalar_tensor_tensor(
                                csj[:], rcq[:, j, 0:1], -1.0, lnj[:],
                                ALU.mult, ALU.mult)
                            otj = otpool.tile([128, 256], b16, tag="otj", name="otj")
                            nc.vector.tensor_scalar_mul(
                                otj[:], negE[:, j, :], csj[:])
                            nc.sync.dma_start(o[pair, :, 4 * qb + j, :], otj[:])
                    return emit

                def fin():
                    r1sq = stat.tile([128, 4], f32, tag="r1sq", name="r1sq")
                    rca = rcq[:, :, 0]
                    nc.vector.tensor_mul(r1sq[:], rca, rca)
                    ssqa = stat.tile([128, 4], f32, tag="ssqa", name="ssqa")
                    nc.vector.tensor_mul(ssqa[:], ssq[:], r1sq[:])
                    lnm = stat.tile([128, 4], f32, tag="lnm", name="lnm")
                    nc.scalar.activation(lnm[:], ssqa[:], AF.Ln,
                                         scale=1.0 / 256.0, bias=eps_t[:])
                    rmst = stat.tile([128, 4], f32, tag="rms", name="rmst")
                    nc.scalar.activation(rmst[:], lnm[:], AF.Exp,
                                         scale=-0.5, bias=lsf_t[:])
                    cs = stat.tile([128, 4], f32, tag="cs", name="cs")
                    nc.vector.scalar_tensor_tensor(
                        cs[:], rca, -1.0, rmst[:], ALU.mult, ALU.mult)
                    ot = otpool.tile([128, 4, 256], b16, tag="ot", name="ot")
                    for j in range(4):
                        nc.vector.tensor_scalar_mul(
                            ot[:, j, :], negE[:, j, :], cs[:, j:j + 1])
                    nc.sync.dma_start(o[pair, :, 4 * qb:4 * qb + 4, :], ot[:])

                for j in range(4):
                    units.append((mkj(j), 2 * (4 * qb + j + 1) * 258 / 2.4))
                if not tail:
                    units.append((fin, 0.0))
                return units

            def merge(a, b):
                """Proportionally interleave unit lists a and b."""
                if not b:
                    return list(a)
                if not a:
                    return list(b)
                out = []
                na, nb_ = len(a), len(b)
                ia = ib = 0
                while ia < na or ib < nb_:
                    # emit from whichever stream is behind its fraction
                    fa = ia / na
                    fb = ib / nb_
                    if ib >= nb_ or (ia < na and fa <= fb):
                        out.append(a[ia]); ia += 1
                    else:
                        out.append(b[ib]); ib += 1
                return out

            # Cross-step PV rebalancing: scores steps alternate ACT-heavy
            # (exp of big superblocks) and PE-heavy (PV of big superblocks).
            # Keep completed steps' PV runs in a pending queue and drain just
            # enough into each scores step to fill the PE while the ACT works;
            # the surplus flows to PE-bound steps and the tail.
            steps = [(0, 1), (0, 0), (0, 2), (0, 3),
                     (1, 1), (1, 0), (1, 2), (1, 3)]
            pending = []  # (step_idx, pe_ns, closure)
            for i, (pair, qb) in enumerate(steps):
                s_units, pt, hi = scores_units(pair, qb)
                qk_pe = sum(pe for _, pe, _ in s_units)
                act = sum(a_ for _, _, a_ in s_units)
                budget = max(0.0, act - qk_pe)
                chosen, got = [], 0.0
                # force-drain old steps (tile-ring reuse), then fill to budget
                while pending and (pending[0][0] <= i - 2 or got < budget):
                    st, pe, cl = pending.pop(0)
                    chosen.append(cl)
                    if st > i - 2:
                        got += pe
                for u in merge([u_[0] for u_ in s_units], chosen):
                    u()
                pending.extend(
                    (i, pe, cl) for cl, pe in
                    pv_units(pair, qb, pt, hi, tail=(i == len(steps) - 1)))
            # tail: round-robin across remaining steps so opsum drains of one
            # qb overlap PV matmuls of another
            from collections import defaultdict as _dd
            by_step = {}
            for st, pe, cl in pending:
                by_step.setdefault(st, []).append(cl)
            qs = list(by_step.values())
            while any(qs):
                for q_ in qs:
                    if q_:
                        q_.pop(0)()

    # Pin Exp+Ln to the one table set containing both
    # (natural_log_exp_and_others) — the greedy per-function chooser otherwise
    # thrashes between exp_and_others and the ln set (~1.3us per reload).
    _orig_gat = bacc.get_activation_tables

    def _gat(arch):
        tabs = _orig_gat(arch)
        for name, fns in tabs.items():
            if name != "natural_log_exp_and_others":
                fns.discard(AF.Exp)
                fns.discard(AF.Ln)
        return tabs

    bacc.get_activation_tables = _gat
    try:
        nc.compile()
    finally:
        bacc.get_activation_tables = _orig_gat
    return nc


def _prep_core_inputs(q, k, v):
    """Host-side shard + layout prep. Returns list of 8 per-core input dicts."""
    in_maps = []
    for c in range(N_CORES):
        b = c // 4
        h0 = 4 * (c % 4)
        # [s, 4, d] -> [4, d, s]
        qs = np.ascontiguousarray(q[b, :, h0:h0 + 4, :].transpose(1, 2, 0))
        ks = np.ascontiguousarray(k[b, :, h0:h0 + 4, :].transpose(1, 2, 0))
        # qkb: [pair, par, p, qT 2048 | kT 2048] bf16
        qkb_ = np.empty((2, 2, 128, 4096), bf16)
        for pair in range(2):
            for par in range(2):
                h = 2 * pair + par
                qkb_[pair, par, :, 0:2048] = qs[h].astype(bf16)
                qkb_[pair, par, :, 2048:4096] = ks[h].astype(bf16)
        # hp32: [pair, p, par, q512 | k512] fp32
        hp32_ = np.empty((2, 128, 2, 1024), np.float32)
        for pair in range(2):
            for par in range(2):
                h = 2 * pair + par
                hp32_[pair, :, par, 0:512] = qs[h][:, :512]
                hp32_[pair, :, par, 512:1024] = ks[h][:, :512]
        vx = np.empty((2, S, 257), np.float32)
        for pair in range(2):
            vx[pair, :, :128] = v[b, :, h0 + 2 * pair, :]
            vx[pair, :, 128:256] = v[b, :, h0 + 2 * pair + 1, :]
            vx[pair, :, 256] = 1.0
        # [2, s, 257] -> partition-major [2, 128, nb, 257]
        vxp = vx.reshape(2, NB, 128, 257).transpose(0, 2, 1, 3)
        vxb_ = np.ascontiguousarray(vxp).astype(bf16)
        vx32_ = np.zeros((2, 128, 4, 260), np.float32)
        vx32_[:, :, :, :257] = vxp[:, :, :4, :]
        in_maps.append({
            "qkb": qkb_, "hp32": hp32_,
            "vxb": vxb_, "vx32": vx32_,
        })
    return in_maps


def kernel(q, k, v, lambda_q1, lambda_k1, lambda_q2, lambda_k2,
           subln_weight, attention_mask):
    global last_results
    from concourse.bass_utils import run_bass_kernel_spmd

    q = np.ascontiguousarray(np.asarray(q, np.float32))
    k = np.ascontiguousarray(np.asarray(k, np.float32))
    v = np.ascontiguousarray(np.asarray(v, np.float32))
    lam1 = np.exp(np.sum(np.asarray(lambda_q1, np.float32)
                         * np.asarray(lambda_k1, np.float32), dtype=np.float32))
    lam2 = np.exp(np.sum(np.asarray(lambda_q2, np.float32)
                         * np.asarray(lambda_k2, np.float32), dtype=np.float32))
    lam_full = np.float32(lam1 - lam2 + np.float32(LAMBDA_INIT))

    key = ("nc", float(lam_full))
    if key not in _CACHE:
        _CACHE[key] = build_nc(float(lam_full))
    nc = _CACHE[key]

    in_maps = _prep_core_inputs(q, k, v)
    trace = bool(int(os.environ.get("KERNEL_TRACE", "0")))
    kw = {}
    if trace:
        kw = dict(trace=True, trace_cores=list(range(N_CORES)))
    res = run_bass_kernel_spmd(nc, in_maps, core_ids=list(range(N_CORES)), **kw)
    last_results = res

    out = np.empty((B, S, N_HEADS // 2, 256), np.float32)
    for c in range(N_CORES):
        b = c // 4
        gp = 2 * (c % 4)
        # o: [pair, p, blk, 256] bf16; seq position = blk*128 + p
        oc = np.asarray(res.results[c]["o"])
        oc = oc.transpose(0, 2, 1, 3).reshape(2, S, 256).astype(np.float32)
        out[b, :, gp, :] = oc[0]
        out[b, :, gp + 1, :] = oc[1]
    out *= np.asarray(subln_weight, np.float32)[None, None, None, :]
    return out


# revision 17
# speedup vs baseline: 1.1554x; 1.1554x over previous
"""Trainium2 Bass kernel for DiffAttention (nn_DiffAttention_49847390437777).

Contract: kernel(**full_inputs) -> full output [2, 2048, 8, 256] fp32.

Sharding (8 cores): core c handles batch b = c//4 and global query-head pairs
{2*(c%4), 2*(c%4)+1} (i.e. heads 4*(c%4)..4*(c%4)+3).  Diff-attention couples
only adjacent head pairs, which stay co-located.  lambda is computed on host
and baked into the program as an immediate; subln_weight is applied on host
after the gather (it multiplies AFTER the RMS norm, so this is exact).

Device algorithm per core (4 heads = 2 pairs, seq 2048, head_dim 128):
  - scores transposed: S^T[k, q] = kT_blk.T @ qT_blk; causal blocks only.
  - softmax without max-subtraction; rowsum folded into the PV matmul via a
    ones-column appended to V (psum col 256).
  - the first 512 q rows run in float32r (cancellation there amplifies bf16
    noise through the RMS norm).  All f32r matmuls keep output free size
    >= 256 so they run at full 1 cycle/row PE speed (the diagonal kb3 block
    is widened from 128 to 256 cols; the extra cols are zeroed in P after).
  - epilogue per 128-row block:  negE = (O2*mu) - O1 with mu = lam*s1/s2
    (one fused scalar_tensor_tensor from PSUM), ssq = sum(negE^2) (fused
    square+row-reduce), then rms scale via ACT Ln/Exp:
       out = negE * (-r1 * exp(-0.5*ln(ssq*r1^2/256 + eps) + ln(1-li)))
  - program order interleaves QK score matmuls of step i+1 with PV matmuls
    of step i so the PE never stalls on the exp drain; the pipeline runs
    across both head pairs (no drain at the pair boundary).  Step order is
    qb = 1,2,3,0 per pair so the fp32 input DMA is off the critical path and
    the final tail (PV of qb0) is the smallest.
  - output is stored bf16 (post-RMS values; rounding is ~0.4% of unit scale)
    and converted to fp32 on host.
"""

import math
import os

import numpy as np
import ml_dtypes

HEAD_DIM = 128
N_HEADS = 16
LAYER_IDX = 12
LAMBDA_INIT = 0.8 - 0.6 * math.exp(-0.3 * (LAYER_IDX - 1))
EPS = 1e-5
SCALE = 1.0 / math.sqrt(HEAD_DIM)
S_FOLD = 1.0 - LAMBDA_INIT

B = 2
S = 2048
NB = S // 128   # 16 key blocks of 128
QB = S // 512   # 4 query superblocks of 512
N_CORES = 8

bf16 = ml_dtypes.bfloat16

_CACHE = {}
last_results = None  # BassKernelResults of the most recent run (for test.py)


def build_nc(lam_full, hiprec=True):
    """Build + compile the per-core Bass program (same program on all cores)."""
    import concourse.bass as bass
    import concourse.mybir as mybir
    import concourse.bacc as bacc
    import concourse.tile as tile
    from concourse.masks import make_upper_triangular
    from contextlib import ExitStack

    f32 = mybir.dt.float32
    f32r = mybir.dt.float32r
    b16 = mybir.dt.bfloat16
    AF = mybir.ActivationFunctionType
    ALU = mybir.AluOpType
    LAM = float(lam_full)

    nc = bacc.Bacc("TRN2", target_bir_lowering=False, debug=False)

    hp32 = nc.dram_tensor("hp32", [2, 128, 2, 1024], f32r, kind="ExternalInput")
    qkb = nc.dram_tensor("qkb", [2, 2, 128, 4096], b16, kind="ExternalInput")
    vxb = nc.dram_tensor("vxb", [2, 128, NB, 257], b16, kind="ExternalInput")
    vx32 = nc.dram_tensor("vx32", [2, 128, 4, 260], f32r, kind="ExternalInput")
    o = nc.dram_tensor("o", [2, 128, NB, 256], b16, kind="ExternalOutput")

    with tile.TileContext(nc) as tc:
        with ExitStack() as ctx:
            ec = ctx.enter_context
            const = ec(tc.tile_pool(name="const", bufs=1))
            qkpool = ec(tc.tile_pool(name="qkpool", bufs=2))
            hppool = ec(tc.tile_pool(name="hppool", bufs=2))
            vpool = ec(tc.tile_pool(name="vpool", bufs=2))
            v32pool = ec(tc.tile_pool(name="v32pool", bufs=2))
            ppool = ec(tc.tile_pool(name="ppool", bufs=3))
            p3pool = ec(tc.tile_pool(name="p3pool", bufs=1))
            negepool = ec(tc.tile_pool(name="negepool", bufs=4))
            stat = ec(tc.tile_pool(name="stat", bufs=4))
            sqpool = ec(tc.tile_pool(name="sqpool", bufs=2))
            otpool = ec(tc.tile_pool(name="otpool", bufs=2))
            spsum = ec(tc.tile_pool(name="spsum", bufs=2, space="PSUM"))
            opsum = ec(tc.tile_pool(name="opsum", bufs=2, space="PSUM"))

            tri16 = const.tile([128, 128], b16)
            make_upper_triangular(nc, tri16[:], val=1.0, diag=True)
            tri32 = const.tile([128, 128], f32)
            make_upper_triangular(nc, tri32[:], val=1.0, diag=True)
            z16 = const.tile([128, 128], b16)
            make_upper_triangular(nc, z16[:], val=0.0, diag=False)
            z32 = const.tile([128, 128], f32)
            make_upper_triangular(nc, z32[:], val=0.0, diag=False)
            eps_t = const.tile([128, 1], f32)
            nc.gpsimd.memset(eps_t[:], EPS)
            lsf_t = const.tile([128, 1], f32)
            nc.gpsimd.memset(lsf_t[:], math.log(S_FOLD))

            # ---- input DMAs, ordered by need ----
            # One tile per DMA chunk: tile-granular dependency tracking means
            # a consumer waits every DMA into its tile, so chunks get own
            # tiles, sized so the first QK only waits ~0.5 MB.
            k_a, k_b, k_c, q_b, q_c = {}, {}, {}, {}, {}
            hp_t, vx_b, vx_3 = {}, {}, {}
            for pair in range(2):
                for par in range(2):   # k blocks 0-3 + its q superblock first
                    k_a[pair, par] = qkpool.tile(
                        [128, 512], b16, tag=f"k_a{par}", name="k_a")
                    nc.sync.dma_start(k_a[pair, par][:], qkb[pair, par, :, 2048:2560])
                    q_b[pair, par] = qkpool.tile(
                        [128, 512], b16, tag=f"q_b{par}", name="q_b")
                    nc.sync.dma_start(q_b[pair, par][:], qkb[pair, par, :, 512:1024])
                for par in range(2):   # fp32 q/k for the qb0 step (2nd step)
                    hp_t[pair, par] = hppool.tile(
                        [128, 1024], f32r, tag=f"hp{par}", name="hp_t")
                    nc.sync.dma_start(hp_t[pair, par][:], hp32[pair, :, par, :])
                vx_b[pair] = vpool.tile([128, NB, 257], b16, tag="vx", name="vx_b")
                nc.sync.dma_start(vx_b[pair][:], vxb[pair])
                for par in range(2):   # k blocks 4-7
                    k_b[pair, par] = qkpool.tile(
                        [128, 512], b16, tag=f"k_b{par}", name="k_b")
                    nc.sync.dma_start(k_b[pair, par][:], qkb[pair, par, :, 2560:3072])
                for par in range(2):   # q cols 1024:2048
                    q_c[pair, par] = qkpool.tile(
                        [128, 1024], b16, tag=f"q_c{par}", name="q_c")
                    nc.sync.dma_start(q_c[pair, par][:], qkb[pair, par, :, 1024:2048])
                for par in range(2):   # k blocks 8-15
                    k_c[pair, par] = qkpool.tile(
                        [128, 1024], b16, tag=f"k_c{par}", name="k_c")
                    nc.sync.dma_start(k_c[pair, par][:], qkb[pair, par, :, 3072:4096])
                vx_3[pair] = v32pool.tile([128, 4, 260], f32r, tag="vx32", name="vx_3")
                nc.sync.dma_start(vx_3[pair][:], vx32[pair])

            def k_src(pair, par, kb):
                if kb < 4:
                    return k_a[pair, par][:], kb * 128
                if kb < 8:
                    return k_b[pair, par][:], (kb - 4) * 128
                return k_c[pair, par][:], (kb - 8) * 128

            def q_src(pair, par, qcol):  # global q column -> (tile, col)
                if qcol < 1024:
                    return q_b[pair, par][:], qcol - 512
                return q_c[pair, par][:], qcol - 1024

            # ---- emission units ----
            def scores_units(pair, qb):
                """List of closures; each emits one (par, 2-kb group):
                QK matmuls -> exp -> tri mask / junk memset."""
                hi = (qb == 0) and hiprec
                q0 = qb * 512
                if hi:
                    pt = {par: p3pool.tile([128, 4, 512], f32r, tag=f"p3_{par}",
                                           name="p3") for par in range(2)}
                else:
                    pt = {par: ppool.tile([128, NB, 512], b16, tag=f"p1_{par}",
                                          name="p1") for par in range(2)}
                ngroups = 2 * qb + 2
                units = []

                def mk(par, g):
                    def emit():
                        dst = pt[par]
                        sp = spsum.tile([128, 2, 512], f32, tag="sp", name="sp")
                        kbs = (2 * g, 2 * g + 1)
                        qoffs = []
                        for t, kb in enumerate(kbs):
                            qoff = 0 if kb < 4 * qb else (kb - 4 * qb) * 128
                            if hi and kb == 3:
                                qoff = 256  # widen to keep f32r at 1 cyc/row
                            qoffs.append(qoff)
                            if hi:
                                kt = hp_t[pair, par][:, 512:1024]
                                ktcol = kb * 128
                                qt = hp_t[pair, par][:, 0:512]
                                qcol = q0 + qoff
                            else:
                                kt, ktcol = k_src(pair, par, kb)
                                qt, qcol = q_src(pair, par, q0 + qoff)
                            nc.tensor.matmul(
                                sp[:, t, qoff:512],
                                kt[:, ktcol:ktcol + 128],
                                qt[:, qcol:qcol + 512 - qoff],
                                start=True, stop=True,
                            )
                        # one exp instr over the group rectangle
                        lo = min(qoffs)
                        nc.scalar.activation(
                            dst[:, kbs[0]:kbs[0] + 2, lo:512],
                            sp[:, 0:2, lo:512],
                            AF.Exp, scale=SCALE,
                        )
                        # junk cols of the rectangle for the 2nd block
                        zt = z32 if hi else z16
                        if qoffs[1] > lo:
                            nc.vector.tensor_copy(
                                dst[:, kbs[1], lo:qoffs[1]],
                                zt[:, 0:qoffs[1] - lo])
                        if hi and kbs[1] == 3:
                            # widened matmul: [256:384) is above-diagonal
                            nc.vector.tensor_copy(
                                dst[:, 3, 256:384], zt[:, 0:128])
                        # triangular mask on diagonal blocks
                        for kb in kbs:
                            if kb >= 4 * qb:
                                dq = q0 + 0  # diag of kb at q col (kb*128)
                                col = (kb - 4 * qb) * 128
                                tri = tri32 if hi else tri16
                                nc.vector.tensor_mul(
                                    dst[:, kb, col:col + 128],
                                    dst[:, kb, col:col + 128], tri[:])
                    return emit

                for g in range(ngroups):
                    for par in range(2):
                        kbs = (2 * g, 2 * g + 1)
                        qoffs = [0 if kb < 4 * qb else
                                 (256 if hi and kb == 3 else (kb - 4 * qb) * 128)
                                 for kb in kbs]
                        pe = sum(512 - qo for qo in qoffs) / 2.4
                        act = 2 * (512 - min(qoffs)) * 0.833 + 265
                        units.append((mk(par, g), pe, act))
                return units, pt, hi

            def pv_units(pair, qb, pt, hi):
                """5 closures: 4 j-runs (PV matmuls + DVE epilogue) + finalize."""
                rcq = stat.tile([128, 4, 2], f32, tag="rcq", name="rcq")
                muq = stat.tile([128, 4], f32, tag="muq", name="muq")
                ssq = stat.tile([128, 4], f32, tag="ssq", name="ssq")
                negE = negepool.tile([128, 4, 256], b16, tag="negE", name="negE")
                vx = vx_3[pair] if hi else vx_b[pair]
                ncols = 258 if hi else 257  # f32r matmul needs even free size
                units = []

                def mkj(j):
                    def emit():
                        jabs = 4 * qb + j
                        opt = opsum.tile([128, 2, 512], f32, tag="op", name="opt")
                        for par in range(2):
                            for kb in range(jabs + 1):
                                nc.tensor.matmul(
                                    opt[:, par, 0:ncols],
                                    pt[par][:, kb, j * 128:(j + 1) * 128],
                                    vx[:, kb, 0:ncols],
                                    start=(kb == 0), stop=(kb == jabs),
                                )
                        # epilogue for this q block
                        nc.vector.reciprocal(rcq[:, j, :], opt[:, :, 256])
                        nc.vector.scalar_tensor_tensor(
                            muq[:, j:j + 1], opt[:, 0, 256:257], LAM,
                            rcq[:, j, 1:2], ALU.mult, ALU.mult)
                        # DVE may read only one non-scalar PSUM input per op
                        t2 = sqpool.tile([128, 256], f32, tag="t2", name="t2")
                        nc.vector.tensor_scalar_mul(
                            t2[:], opt[:, 1, 0:256], muq[:, j:j + 1])
                        nc.vector.tensor_sub(
                            negE[:, j, :], t2[:], opt[:, 0, 0:256])
                        sq = sqpool.tile([128, 256], b16, tag="sq", name="sq")
                        nc.vector.scalar_tensor_tensor(
                            sq[:], negE[:, j, :], 1.0, negE[:, j, :],
                            ALU.mult, ALU.mult, accum_out=ssq[:, j:j + 1])
                    return emit

                def fin():
                    r1sq = stat.tile([128, 4], f32, tag="r1sq", name="r1sq")
                    rca = rcq[:, :, 0]
                    nc.vector.tensor_mul(r1sq[:], rca, rca)
                    ssqa = stat.tile([128, 4], f32, tag="ssqa", name="ssqa")
                    nc.vector.tensor_mul(ssqa[:], ssq[:], r1sq[:])
                    lnm = stat.tile([128, 4], f32, tag="lnm", name="lnm")
                    nc.scalar.activation(lnm[:], ssqa[:], AF.Ln,
                                         scale=1.0 / 256.0, bias=eps_t[:])
                    rmst = stat.tile([128, 4], f32, tag="rms", name="rmst")
                    nc.scalar.activation(rmst[:], lnm[:], AF.Exp,
                                         scale=-0.5, bias=lsf_t[:])
                    cs = stat.tile([128, 4], f32, tag="cs", name="cs")
                    nc.vector.scalar_tensor_tensor(
                        cs[:], rca, -1.0, rmst[:], ALU.mult, ALU.mult)
                    ot = otpool.tile([128, 4, 256], b16, tag="ot", name="ot")
                    for j in range(4):
                        nc.vector.tensor_scalar_mul(
                            ot[:, j, :], negE[:, j, :], cs[:, j:j + 1])
                    nc.sync.dma_start(o[pair, :, 4 * qb:4 * qb + 4, :], ot[:])

                for j in range(4):
                    units.append((mkj(j), 2 * (4 * qb + j + 1) * 258 / 2.4))
                units.append((fin, 0.0))
                return units

            def merge(a, b):
                """Proportionally interleave unit lists a and b."""
                if not b:
                    return list(a)
                if not a:
                    return list(b)
                out = []
                na, nb_ = len(a), len(b)
                ia = ib = 0
                while ia < na or ib < nb_:
                    # emit from whichever stream is behind its fraction
                    fa = ia / na
                    fb = ib / nb_
                    if ib >= nb_ or (ia < na and fa <= fb):
                        out.append(a[ia]); ia += 1
                    else:
                        out.append(b[ib]); ib += 1
                return out

            # Cross-step PV rebalancing: scores steps alternate ACT-heavy
            # (exp of big superblocks) and PE-heavy (PV of big superblocks).
            # Keep completed steps' PV runs in a pending queue and drain just
            # enough into each scores step to fill the PE while the ACT works;
            # the surplus flows to PE-bound steps and the tail.
            steps = [(0, 1), (0, 0), (0, 2), (0, 3),
                     (1, 1), (1, 0), (1, 2), (1, 3)]
            pending = []  # (step_idx, pe_ns, closure)
            for i, (pair, qb) in enumerate(steps):
                s_units, pt, hi = scores_units(pair, qb)
                qk_pe = sum(pe for _, pe, _ in s_units)
                act = sum(a_ for _, _, a_ in s_units)
                budget = max(0.0, act - qk_pe)
                chosen, got = [], 0.0
                # force-drain old steps (tile-ring reuse), then fill to budget
                while pending and (pending[0][0] <= i - 2 or got < budget):
                    st, pe, cl = pending.pop(0)
                    chosen.append(cl)
                    if st > i - 2:
                        got += pe
                for u in merge([u_[0] for u_ in s_units], chosen):
                    u()
                pending.extend(
                    (i, pe, cl) for cl, pe in pv_units(pair, qb, pt, hi))
            # tail: round-robin across remaining steps so opsum drains of one
            # qb overlap PV matmuls of another
            from collections import defaultdict as _dd
            by_step = {}
            for st, pe, cl in pending:
                by_step.setdefault(st, []).append(cl)
            qs = list(by_step.values())
            while any(qs):
                for q_ in qs:
                    if q_:
                        q_.pop(0)()

    # Pin Exp+Ln to the one table set containing both
    # (natural_log_exp_and_others) — the greedy per-function chooser otherwise
    # thrashes between exp_and_others and the ln set (~1.3us per reload).
    _orig_gat = bacc.get_activation_tables

    def _gat(arch):
        tabs = _orig_gat(arch)
        for name, fns in tabs.items():
            if name != "natural_log_exp_and_others":
                fns.discard(AF.Exp)
                fns.discard(AF.Ln)
        return tabs

    bacc.get_activation_tables = _gat
    try:
        nc.compile()
    finally:
        bacc.get_activation_tables = _orig_gat
    return nc


def _prep_core_inputs(q, k, v):
    """Host-side shard + layout prep. Returns list of 8 per-core input dicts."""
    in_maps = []
    for c in range(N_CORES):
        b = c // 4
        h0 = 4 * (c % 4)
        # [s, 4, d] -> [4, d, s]
        qs = np.ascontiguousarray(q[b, :, h0:h0 + 4, :].transpose(1, 2, 0))
        ks = np.ascontiguousarray(k[b, :, h0:h0 + 4, :].transpose(1, 2, 0))
        # qkb: [pair, par, p, qT 2048 | kT 2048] bf16
        qkb_ = np.empty((2, 2, 128, 4096), bf16)
        for pair in range(2):
            for par in range(2):
                h = 2 * pair + par
                qkb_[pair, par, :, 0:2048] = qs[h].astype(bf16)
                qkb_[pair, par, :, 2048:4096] = ks[h].astype(bf16)
        # hp32: [pair, p, par, q512 | k512] fp32
        hp32_ = np.empty((2, 128, 2, 1024), np.float32)
        for pair in range(2):
            for par in range(2):
                h = 2 * pair + par
                hp32_[pair, :, par, 0:512] = qs[h][:, :512]
                hp32_[pair, :, par, 512:1024] = ks[h][:, :512]
        vx = np.empty((2, S, 257), np.float32)
        for pair in range(2):
            vx[pair, :, :128] = v[b, :, h0 + 2 * pair, :]
            vx[pair, :, 128:256] = v[b, :, h0 + 2 * pair + 1, :]
            vx[pair, :, 256] = 1.0
        # [2, s, 257] -> partition-major [2, 128, nb, 257]
        vxp = vx.reshape(2, NB, 128, 257).transpose(0, 2, 1, 3)
        vxb_ = np.ascontiguousarray(vxp).astype(bf16)
        vx32_ = np.zeros((2, 128, 4, 260), np.float32)
        vx32_[:, :, :, :257] = vxp[:, :, :4, :]
        in_maps.append({
            "qkb": qkb_, "hp32": hp32_,
            "vxb": vxb_, "vx32": vx32_,
        })
    return in_maps


def kernel(q, k, v, lambda_q1, lambda_k1, lambda_q2, lambda_k2,
           subln_weight, attention_mask):
    global last_results
    from concourse.bass_utils import run_bass_kernel_spmd

    q = np.ascontiguousarray(np.asarray(q, np.float32))
    k = np.ascontiguousarray(np.asarray(k, np.float32))
    v = np.ascontiguousarray(np.asarray(v, np.float32))
    lam1 = np.exp(np.sum(np.asarray(lambda_q1, np.float32)
                         * np.asarray(lambda_k1, np.float32), dtype=np.float32))
    lam2 = np.exp(np.sum(np.asarray(lambda_q2, np.float32)
                         * np.asarray(lambda_k2, np.float32), dtype=np.float32))
    lam_full = np.float32(lam1 - lam2 + np.float32(LAMBDA_INIT))

    key = ("nc", float(lam_full))
    if key not in _CACHE:
        _CACHE[key] = build_nc(float(lam_full))
    nc = _CACHE[key]

    in_maps = _prep_core_inputs(q, k, v)
    trace = bool(int(os.environ.get("KERNEL_TRACE", "0")))
    kw = {}
    if trace:
        kw = dict(trace=True, trace_cores=list(range(N_CORES)))
    res = run_bass_kernel_spmd(nc, in_maps, core_ids=list(range(N_CORES)), **kw)
    last_results = res

    out = np.empty((B, S, N_HEADS // 2, 256), np.float32)
    for c in range(N_CORES):
        b = c // 4
        gp = 2 * (c % 4)
        # o: [pair, p, blk, 256] bf16; seq position = blk*128 + p
        oc = np.asarray(res.results[c]["o"])
        oc = oc.transpose(0, 2, 1, 3).reshape(2, S, 256).astype(np.float32)
        out[b, :, gp, :] = oc[0]
        out[b, :, gp + 1, :] = oc[1]
    out *= np.asarray(subln_weight, np.float32)[None, None, None, :]
    return out


# revision 18
# speedup vs baseline: 1.1677x; 1.0107x over previous
"""Trainium2 Bass kernel for DiffAttention (nn_DiffAttention_49847390437777).

Contract: kernel(**full_inputs) -> full output [2, 2048, 8, 256] fp32.

Sharding (8 cores): core c handles batch b = c//4 and global query-head pairs
{2*(c%4), 2*(c%4)+1} (i.e. heads 4*(c%4)..4*(c%4)+3).  Diff-attention couples
only adjacent head pairs, which stay co-located.  lambda is computed on host
and baked into the program as an immediate; subln_weight is applied on host
after the gather (it multiplies AFTER the RMS norm, so this is exact).

Device algorithm per core (4 heads = 2 pairs, seq 2048, head_dim 128):
  - scores transposed: S^T[k, q] = kT_blk.T @ qT_blk; causal blocks only.
  - softmax without max-subtraction; rowsum folded into the PV matmul via a
    ones-column appended to V (psum col 256).
  - the first 512 q rows run in float32r (cancellation there amplifies bf16
    noise through the RMS norm).  All f32r matmuls keep output free size
    >= 256 so they run at full 1 cycle/row PE speed (the diagonal kb3 block
    is widened from 128 to 256 cols; the extra cols are zeroed in P after).
  - epilogue per 128-row block:  negE = (O2*mu) - O1 with mu = lam*s1/s2
    (one fused scalar_tensor_tensor from PSUM), ssq = sum(negE^2) (fused
    square+row-reduce), then rms scale via ACT Ln/Exp:
       out = negE * (-r1 * exp(-0.5*ln(ssq*r1^2/256 + eps) + ln(1-li)))
  - program order interleaves QK score matmuls of step i+1 with PV matmuls
    of step i so the PE never stalls on the exp drain; the pipeline runs
    across both head pairs (no drain at the pair boundary).  Step order is
    qb = 1,2,3,0 per pair so the fp32 input DMA is off the critical path and
    the final tail (PV of qb0) is the smallest.
  - output is stored bf16 (post-RMS values; rounding is ~0.4% of unit scale)
    and converted to fp32 on host.
"""

import math
import os

import numpy as np
import ml_dtypes

HEAD_DIM = 128
N_HEADS = 16
LAYER_IDX = 12
LAMBDA_INIT = 0.8 - 0.6 * math.exp(-0.3 * (LAYER_IDX - 1))
EPS = 1e-5
SCALE = 1.0 / math.sqrt(HEAD_DIM)
S_FOLD = 1.0 - LAMBDA_INIT

B = 2
S = 2048
NB = S // 128   # 16 key blocks of 128
QB = S // 512   # 4 query superblocks of 512
N_CORES = 8

bf16 = ml_dtypes.bfloat16

_CACHE = {}
last_results = None  # BassKernelResults of the most recent run (for test.py)


def build_nc(lam_full, hiprec=True):
    """Build + compile the per-core Bass program (same program on all cores)."""
    import concourse.bass as bass
    import concourse.mybir as mybir
    import concourse.bacc as bacc
    import concourse.tile as tile
    from concourse.masks import make_upper_triangular
    from contextlib import ExitStack

    f32 = mybir.dt.float32
    f32r = mybir.dt.float32r
    b16 = mybir.dt.bfloat16
    AF = mybir.ActivationFunctionType
    ALU = mybir.AluOpType
    LAM = float(lam_full)

    nc = bacc.Bacc("TRN2", target_bir_lowering=False, debug=False)

    hp32 = nc.dram_tensor("hp32", [2, 128, 2, 1024], f32r, kind="ExternalInput")
    qkb = nc.dram_tensor("qkb", [2, 2, 128, 4096], b16, kind="ExternalInput")
    vxb = nc.dram_tensor("vxb", [2, 128, NB, 257], b16, kind="ExternalInput")
    vx32 = nc.dram_tensor("vx32", [2, 128, 4, 260], f32r, kind="ExternalInput")
    o = nc.dram_tensor("o", [2, 128, NB, 256], b16, kind="ExternalOutput")

    with tile.TileContext(nc) as tc:
        with ExitStack() as ctx:
            ec = ctx.enter_context
            const = ec(tc.tile_pool(name="const", bufs=1))
            qkpool = ec(tc.tile_pool(name="qkpool", bufs=2))
            hppool = ec(tc.tile_pool(name="hppool", bufs=2))
            vpool = ec(tc.tile_pool(name="vpool", bufs=2))
            v32pool = ec(tc.tile_pool(name="v32pool", bufs=2))
            ppool = ec(tc.tile_pool(name="ppool", bufs=3))
            p3pool = ec(tc.tile_pool(name="p3pool", bufs=1))
            negepool = ec(tc.tile_pool(name="negepool", bufs=4))
            stat = ec(tc.tile_pool(name="stat", bufs=4))
            sqpool = ec(tc.tile_pool(name="sqpool", bufs=2))
            otpool = ec(tc.tile_pool(name="otpool", bufs=2))
            spsum = ec(tc.tile_pool(name="spsum", bufs=2, space="PSUM"))
            opsum = ec(tc.tile_pool(name="opsum", bufs=2, space="PSUM"))

            tri16 = const.tile([128, 128], b16)
            make_upper_triangular(nc, tri16[:], val=1.0, diag=True)
            tri32 = const.tile([128, 128], f32)
            make_upper_triangular(nc, tri32[:], val=1.0, diag=True)
            z16 = const.tile([128, 128], b16)
            make_upper_triangular(nc, z16[:], val=0.0, diag=False)
            z32 = const.tile([128, 128], f32)
            make_upper_triangular(nc, z32[:], val=0.0, diag=False)
            eps_t = const.tile([128, 1], f32)
            nc.gpsimd.memset(eps_t[:], EPS)
            lsf_t = const.tile([128, 1], f32)
            nc.gpsimd.memset(lsf_t[:], math.log(S_FOLD))

            # ---- input DMAs, ordered by need ----
            # One tile per DMA chunk: tile-granular dependency tracking means
            # a consumer waits every DMA into its tile, so chunks get own
            # tiles, sized so the first QK only waits ~0.5 MB.
            k_a, k_b, k_c, q_b, q_c = {}, {}, {}, {}, {}
            hp_t, vx_b, vx_3 = {}, {}, {}
            for pair in range(2):
                for par in range(2):   # k blocks 0-3 + its q superblock first
                    k_a[pair, par] = qkpool.tile(
                        [128, 512], b16, tag=f"k_a{par}", name="k_a")
                    nc.sync.dma_start(k_a[pair, par][:], qkb[pair, par, :, 2048:2560])
                    q_b[pair, par] = qkpool.tile(
                        [128, 512], b16, tag=f"q_b{par}", name="q_b")
                    nc.sync.dma_start(q_b[pair, par][:], qkb[pair, par, :, 512:1024])
                for par in range(2):   # fp32 q/k for the qb0 step (2nd step)
                    hp_t[pair, par] = hppool.tile(
                        [128, 1024], f32r, tag=f"hp{par}", name="hp_t")
                    nc.sync.dma_start(hp_t[pair, par][:], hp32[pair, :, par, :])
                vx_b[pair] = vpool.tile([128, NB, 257], b16, tag="vx", name="vx_b")
                nc.sync.dma_start(vx_b[pair][:], vxb[pair])
                for par in range(2):   # k blocks 4-7
                    k_b[pair, par] = qkpool.tile(
                        [128, 512], b16, tag=f"k_b{par}", name="k_b")
                    nc.sync.dma_start(k_b[pair, par][:], qkb[pair, par, :, 2560:3072])
                for par in range(2):   # q cols 1024:2048
                    q_c[pair, par] = qkpool.tile(
                        [128, 1024], b16, tag=f"q_c{par}", name="q_c")
                    nc.sync.dma_start(q_c[pair, par][:], qkb[pair, par, :, 1024:2048])
                for par in range(2):   # k blocks 8-15
                    k_c[pair, par] = qkpool.tile(
                        [128, 1024], b16, tag=f"k_c{par}", name="k_c")
                    nc.sync.dma_start(k_c[pair, par][:], qkb[pair, par, :, 3072:4096])
                vx_3[pair] = v32pool.tile([128, 4, 260], f32r, tag="vx32", name="vx_3")
                nc.sync.dma_start(vx_3[pair][:], vx32[pair])

            def k_src(pair, par, kb):
                if kb < 4:
                    return k_a[pair, par][:], kb * 128
                if kb < 8:
                    return k_b[pair, par][:], (kb - 4) * 128
                return k_c[pair, par][:], (kb - 8) * 128

            def q_src(pair, par, qcol):  # global q column -> (tile, col)
                if qcol < 1024:
                    return q_b[pair, par][:], qcol - 512
                return q_c[pair, par][:], qcol - 1024

            # ---- emission units ----
            def scores_units(pair, qb):
                """List of closures; each emits one (par, 2-kb group):
                QK matmuls -> exp -> tri mask / junk memset."""
                hi = (qb == 0) and hiprec
                q0 = qb * 512
                if hi:
                    pt = {par: p3pool.tile([128, 4, 512], f32r, tag=f"p3_{par}",
                                           name="p3") for par in range(2)}
                else:
                    pt = {par: ppool.tile([128, NB, 512], b16, tag=f"p1_{par}",
                                          name="p1") for par in range(2)}
                ngroups = 2 * qb + 2
                units = []

                def mk(par, g):
                    def emit():
                        dst = pt[par]
                        sp = spsum.tile([128, 2, 512], f32, tag="sp", name="sp")
                        kbs = (2 * g, 2 * g + 1)
                        qoffs = []
                        for t, kb in enumerate(kbs):
                            qoff = 0 if kb < 4 * qb else (kb - 4 * qb) * 128
                            if hi and kb == 3:
                                qoff = 256  # widen to keep f32r at 1 cyc/row
                            qoffs.append(qoff)
                            if hi:
                                kt = hp_t[pair, par][:, 512:1024]
                                ktcol = kb * 128
                                qt = hp_t[pair, par][:, 0:512]
                                qcol = q0 + qoff
                            else:
                                kt, ktcol = k_src(pair, par, kb)
                                qt, qcol = q_src(pair, par, q0 + qoff)
                            nc.tensor.matmul(
                                sp[:, t, qoff:512],
                                kt[:, ktcol:ktcol + 128],
                                qt[:, qcol:qcol + 512 - qoff],
                                start=True, stop=True,
                            )
                        # one exp instr over the group rectangle
                        lo = min(qoffs)
                        nc.scalar.activation(
                            dst[:, kbs[0]:kbs[0] + 2, lo:512],
                            sp[:, 0:2, lo:512],
                            AF.Exp, scale=SCALE,
                        )
                        # junk cols of the rectangle for the 2nd block
                        zt = z32 if hi else z16
                        if qoffs[1] > lo:
                            nc.vector.tensor_copy(
                                dst[:, kbs[1], lo:qoffs[1]],
                                zt[:, 0:qoffs[1] - lo])
                        if hi and kbs[1] == 3:
                            # widened matmul: [256:384) is above-diagonal
                            nc.vector.tensor_copy(
                                dst[:, 3, 256:384], zt[:, 0:128])
                        # triangular mask on diagonal blocks
                        for kb in kbs:
                            if kb >= 4 * qb:
                                col = (kb - 4 * qb) * 128
                                tri = tri32 if hi else tri16
                                nc.vector.tensor_mul(
                                    dst[:, kb, col:col + 128],
                                    dst[:, kb, col:col + 128], tri[:])
                    return emit

                for g in range(ngroups):
                    for par in range(2):
                        kbs = (2 * g, 2 * g + 1)
                        qoffs = [0 if kb < 4 * qb else
                                 (256 if hi and kb == 3 else (kb - 4 * qb) * 128)
                                 for kb in kbs]
                        pe = sum(512 - qo for qo in qoffs) / 2.4
                        act = 2 * (512 - min(qoffs)) * 0.833 + 265
                        units.append((mk(par, g), pe, act))
                return units, pt, hi

            def pv_units(pair, qb, pt, hi):
                """5 closures: 4 j-runs (PV matmuls + DVE epilogue) + finalize."""
                rcq = stat.tile([128, 4, 2], f32, tag="rcq", name="rcq")
                muq = stat.tile([128, 4], f32, tag="muq", name="muq")
                ssq = stat.tile([128, 4], f32, tag="ssq", name="ssq")
                negE = negepool.tile([128, 4, 256], b16, tag="negE", name="negE")
                vx = vx_3[pair] if hi else vx_b[pair]
                ncols = 258 if hi else 257  # f32r matmul needs even free size
                units = []

                def mkj(j):
                    def emit():
                        jabs = 4 * qb + j
                        opt = opsum.tile([128, 2, 512], f32, tag="op", name="opt")
                        for par in range(2):
                            for kb in range(jabs + 1):
                                nc.tensor.matmul(
                                    opt[:, par, 0:ncols],
                                    pt[par][:, kb, j * 128:(j + 1) * 128],
                                    vx[:, kb, 0:ncols],
                                    start=(kb == 0), stop=(kb == jabs),
                                )
                        # epilogue for this q block
                        nc.vector.reciprocal(rcq[:, j, :], opt[:, :, 256])
                        nc.vector.scalar_tensor_tensor(
                            muq[:, j:j + 1], opt[:, 0, 256:257], LAM,
                            rcq[:, j, 1:2], ALU.mult, ALU.mult)
                        # DVE may read only one non-scalar PSUM input per op
                        t2 = sqpool.tile([128, 256], f32, tag="t2", name="t2")
                        nc.vector.tensor_scalar_mul(
                            t2[:], opt[:, 1, 0:256], muq[:, j:j + 1])
                        nc.vector.tensor_sub(
                            negE[:, j, :], t2[:], opt[:, 0, 0:256])
                        sq = sqpool.tile([128, 256], b16, tag="sq", name="sq")
                        nc.vector.scalar_tensor_tensor(
                            sq[:], negE[:, j, :], 1.0, negE[:, j, :],
                            ALU.mult, ALU.mult, accum_out=ssq[:, j:j + 1])
                    return emit

                def fin():
                    r1sq = stat.tile([128, 4], f32, tag="r1sq", name="r1sq")
                    rca = rcq[:, :, 0]
                    nc.vector.tensor_mul(r1sq[:], rca, rca)
                    ssqa = stat.tile([128, 4], f32, tag="ssqa", name="ssqa")
                    nc.vector.tensor_mul(ssqa[:], ssq[:], r1sq[:])
                    lnm = stat.tile([128, 4], f32, tag="lnm", name="lnm")
                    nc.scalar.activation(lnm[:], ssqa[:], AF.Ln,
                                         scale=1.0 / 256.0, bias=eps_t[:])
                    rmst = stat.tile([128, 4], f32, tag="rms", name="rmst")
                    nc.scalar.activation(rmst[:], lnm[:], AF.Exp,
                                         scale=-0.5, bias=lsf_t[:])
                    cs = stat.tile([128, 4], f32, tag="cs", name="cs")
                    nc.vector.scalar_tensor_tensor(
                        cs[:], rca, -1.0, rmst[:], ALU.mult, ALU.mult)
                    ot = otpool.tile([128, 4, 256], b16, tag="ot", name="ot")
                    for j in range(4):
                        nc.vector.tensor_scalar_mul(
                            ot[:, j, :], negE[:, j, :], cs[:, j:j + 1])
                    nc.sync.dma_start(o[pair, :, 4 * qb:4 * qb + 4, :], ot[:])

                for j in range(4):
                    units.append((mkj(j), 2 * (4 * qb + j + 1) * 258 / 2.4))
                units.append((fin, 0.0))
                return units

            def merge(a, b):
                """Proportionally interleave unit lists a and b."""
                if not b:
                    return list(a)
                if not a:
                    return list(b)
                out = []
                na, nb_ = len(a), len(b)
                ia = ib = 0
                while ia < na or ib < nb_:
                    # emit from whichever stream is behind its fraction
                    fa = ia / na
                    fb = ib / nb_
                    if ib >= nb_ or (ia < na and fa <= fb):
                        out.append(a[ia]); ia += 1
                    else:
                        out.append(b[ib]); ib += 1
                return out

            # Cross-step PV rebalancing: scores steps alternate ACT-heavy
            # (exp of big superblocks) and PE-heavy (PV of big superblocks).
            # Keep completed steps' PV runs in a pending queue and drain just
            # enough into each scores step to fill the PE while the ACT works;
            # the surplus flows to PE-bound steps and the tail.
            steps = [(0, 1), (0, 0), (0, 2), (0, 3),
                     (1, 1), (1, 0), (1, 2), (1, 3)]
            pending = []  # (step_idx, pe_ns, closure)
            for i, (pair, qb) in enumerate(steps):
                s_units, pt, hi = scores_units(pair, qb)
                qk_pe = sum(pe for _, pe, _ in s_units)
                act = sum(a_ for _, _, a_ in s_units)
                budget = max(0.0, act - qk_pe)
                chosen, got = [], 0.0
                # force-drain old steps (tile-ring reuse), then fill to budget
                while pending and (pending[0][0] <= i - 2 or got < budget):
                    st, pe, cl = pending.pop(0)
                    chosen.append(cl)
                    if st > i - 2:
                        got += pe
                for u in merge([u_[0] for u_ in s_units], chosen):
                    u()
                pending.extend(
                    (i, pe, cl) for cl, pe in pv_units(pair, qb, pt, hi))
            # tail: round-robin across remaining steps so opsum drains of one
            # qb overlap PV matmuls of another
            by_step = {}
            for st, pe, cl in pending:
                by_step.setdefault(st, []).append(cl)
            qs = list(by_step.values())
            while any(qs):
                for q_ in qs:
                    if q_:
                        q_.pop(0)()

    # Pin Exp+Ln to the one table set containing both
    # (natural_log_exp_and_others) — the greedy per-function chooser otherwise
    # thrashes between exp_and_others and the ln set (~1.3us per reload).
    _orig_gat = bacc.get_activation_tables

    def _gat(arch):
        tabs = _orig_gat(arch)
        for name, fns in tabs.items():
            if name != "natural_log_exp_and_others":
                fns.discard(AF.Exp)
                fns.discard(AF.Ln)
        return tabs

    bacc.get_activation_tables = _gat
    try:
        nc.compile()
    finally:
        bacc.get_activation_tables = _orig_gat
    return nc


def _prep_core_inputs(q, k, v):
    """Host-side shard + layout prep. Returns list of 8 per-core input dicts."""
    in_maps = []
    for c in range(N_CORES):
        b = c // 4
        h0 = 4 * (c % 4)
        # [s, 4, d] -> [4, d, s]
        qs = np.ascontiguousarray(q[b, :, h0:h0 + 4, :].transpose(1, 2, 0))
        ks = np.ascontiguousarray(k[b, :, h0:h0 + 4, :].transpose(1, 2, 0))
        # qkb: [pair, par, p, qT 2048 | kT 2048] bf16
        qkb_ = np.empty((2, 2, 128, 4096), bf16)
        for pair in range(2):
            for par in range(2):
                h = 2 * pair + par
                qkb_[pair, par, :, 0:2048] = qs[h].astype(bf16)
                qkb_[pair, par, :, 2048:4096] = ks[h].astype(bf16)
        # hp32: [pair, p, par, q512 | k512] fp32
        hp32_ = np.empty((2, 128, 2, 1024), np.float32)
        for pair in range(2):
            for par in range(2):
                h = 2 * pair + par
                hp32_[pair, :, par, 0:512] = qs[h][:, :512]
                hp32_[pair, :, par, 512:1024] = ks[h][:, :512]
        vx = np.empty((2, S, 257), np.float32)
        for pair in range(2):
            vx[pair, :, :128] = v[b, :, h0 + 2 * pair, :]
            vx[pair, :, 128:256] = v[b, :, h0 + 2 * pair + 1, :]
            vx[pair, :, 256] = 1.0
        # [2, s, 257] -> partition-major [2, 128, nb, 257]
        vxp = vx.reshape(2, NB, 128, 257).transpose(0, 2, 1, 3)
        vxb_ = np.ascontiguousarray(vxp).astype(bf16)
        vx32_ = np.zeros((2, 128, 4, 260), np.float32)
        vx32_[:, :, :, :257] = vxp[:, :, :4, :]
        in_maps.append({
            "qkb": qkb_, "hp32": hp32_,
            "vxb": vxb_, "vx32": vx32_,
        })
    return in_maps


def kernel(q, k, v, lambda_q1, lambda_k1, lambda_q2, lambda_k2,
           subln_weight, attention_mask):
    global last_results
    from concourse.bass_utils import run_bass_kernel_spmd

    q = np.ascontiguousarray(np.asarray(q, np.float32))
    k = np.ascontiguousarray(np.asarray(k, np.float32))
    v = np.ascontiguousarray(np.asarray(v, np.float32))
    lam1 = np.exp(np.sum(np.asarray(lambda_q1, np.float32)
                         * np.asarray(lambda_k1, np.float32), dtype=np.float32))
    lam2 = np.exp(np.sum(np.asarray(lambda_q2, np.float32)
                         * np.asarray(lambda_k2, np.float32), dtype=np.float32))
    lam_full = np.float32(lam1 - lam2 + np.float32(LAMBDA_INIT))

    key = ("nc", float(lam_full))
    if key not in _CACHE:
        _CACHE[key] = build_nc(float(lam_full))
    nc = _CACHE[key]

    in_maps = _prep_core_inputs(q, k, v)
    trace = bool(int(os.environ.get("KERNEL_TRACE", "0")))
    kw = {}
    if trace:
        kw = dict(trace=True, trace_cores=list(range(N_CORES)))
    res = run_bass_kernel_spmd(nc, in_maps, core_ids=list(range(N_CORES)), **kw)
    last_results = res

    out = np.empty((B, S, N_HEADS // 2, 256), np.float32)
    for c in range(N_CORES):
        b = c // 4
        gp = 2 * (c % 4)
        # o: [pair, p, blk, 256] bf16; seq position = blk*128 + p
        oc = np.asarray(res.results[c]["o"])
        oc = oc.transpose(0, 2, 1, 3).reshape(2, S, 256).astype(np.float32)
        out[b, :, gp, :] = oc[0]
        out[b, :, gp + 1, :] = oc[1]
    out *= np.asarray(subln_weight, np.float32)[None, None, None, :]
    return out
